# revision 27
# baseline (speedup 1.0000x reference)
"""Multi-head attention (B=4, S=2048, D=1024, H=16, hd=64) on 8 trn2 cores.

Sharding: core c -> batch b = c//2, head-group g = c%2 (8 heads = 512 proj dims).
Each core computes QKV projections for its batch with head-sliced weights,
full attention for its 8 heads (attn weights are part of the output), and a
partial output projection. Host sums the two partial outputs per batch and
concatenates attention slices.

All matmuls run in float32r (TF32-like e8m11, full PE rate). Softmax
normalization is folded into the ACT exp as a per-partition -ln(rowsum) bias;
rowsums come from a ones-row appended to V in the transposed-orientation
context matmul.
"""

import sys

if "/opt/trn_rl_repo" not in sys.path:
    sys.path.insert(0, "/opt/trn_rl_repo")

import numpy as np

import concourse.bass as bass
import concourse.mybir as mybir
import concourse.tile as tile
from concourse import bacc
from concourse.bass_utils import run_bass_kernel_spmd

FP32 = mybir.dt.float32
FP32R = mybir.dt.float32r

B, S, D = 4, 2048, 1024
NH, HD = 16, 64
HG = 8  # heads per core
PJ = HG * HD  # 512 proj dims per core
P = 128
NCORES = 8

Exp = mybir.ActivationFunctionType.Exp
Ln = mybir.ActivationFunctionType.Ln
ADD = mybir.AluOpType.add
MULT = mybir.AluOpType.mult


def round_fp32r(x: np.ndarray) -> np.ndarray:
    """Round fp32 to fp32r (e8m11, RNE) — what the HW datapath expects.

    uint32 arithmetic wraps only for NaN-pattern inputs (not produced here).
    """
    u = np.ascontiguousarray(x).view(np.uint32)
    r = (u + np.uint32(0x7FF) + ((u >> np.uint32(12)) & np.uint32(1))) & np.uint32(
        0xFFFFF000
    )
    return r.view(np.float32)


def build_nc():
    nc = bacc.Bacc(None, target_bir_lowering=False, debug=False)

    xq = nc.dram_tensor("xq", [D, S], FP32R, kind="ExternalInput")
    xk = nc.dram_tensor("xk", [D, S], FP32R, kind="ExternalInput")
    xv = nc.dram_tensor("xv", [D, S], FP32R, kind="ExternalInput")
    wq = nc.dram_tensor("wq", [D, PJ], FP32R, kind="ExternalInput")
    wk = nc.dram_tensor("wk", [D, PJ], FP32R, kind="ExternalInput")
    wv = nc.dram_tensor("wv", [D, PJ], FP32R, kind="ExternalInput")
    wo = nc.dram_tensor("wo", [PJ, D], FP32R, kind="ExternalInput")
    bq = nc.dram_tensor("bq", [PJ, 1], FP32, kind="ExternalInput")
    bk = nc.dram_tensor("bk", [PJ, 1], FP32, kind="ExternalInput")
    bv = nc.dram_tensor("bv", [1, PJ], FP32R, kind="ExternalInput")
    ident = nc.dram_tensor("ident", [P, P], FP32, kind="ExternalInput")

    attn_o = nc.dram_tensor("attn_o", [HG, S, S], FP32, kind="ExternalOutput")
    out_p = nc.dram_tensor("out_p", [S, D], FP32, kind="ExternalOutput")

    with tile.TileContext(nc) as tc:
        with (
            tc.tile_pool(name="persist", bufs=1) as persist,
            tc.tile_pool(name="xw", bufs=8) as xw,
            tc.tile_pool(name="work", bufs=2) as work,
            tc.tile_pool(name="small", bufs=2) as small,
            tc.tile_pool(name="psum_mm", bufs=3, space="PSUM") as psum_mm,
            tc.tile_pool(name="psum_ctx", bufs=1, space="PSUM") as psum_ctx,
        ):
            # ---- persistent SBUF tensors ----
            qt_t = [
                persist.tile([P, S], FP32R, tag=f"qt{i}", name=f"qt{i}")
                for i in range(4)
            ]
            kt_t = [
                persist.tile([P, S], FP32R, tag=f"kt{i}", name=f"kt{i}")
                for i in range(4)
            ]
            # V with a ones column per head: (128, 8, 65)
            v_t = [
                persist.tile([P, HG, HD + 1], FP32R, tag=f"v{i}", name=f"v{i}")
                for i in range(16)
            ]
            ct_t = [
                persist.tile([P, S], FP32R, tag=f"ct{i}", name=f"ct{i}")
                for i in range(4)
            ]
            bias_qk = persist.tile([P, 8], FP32, tag="bias_qk")  # bq 0:4, bk 4:8
            bv_row = persist.tile([1, PJ], FP32R, tag="bv_row")
            # ones (fp32r) usable from base partitions 0 and 64
            ones_t = persist.tile([P, P], FP32R, tag="ones_t")

            ident_t = persist.tile([P, P], FP32, tag="ident_t")
            nc.sync.dma_start(ident_t[:], ident[:])
            ones_f = small.tile([P, P], FP32, tag="ones_f", bufs=1)
            nc.vector.memset(ones_f[:], 1.0)
            nc.vector.tensor_copy(ones_t[:], ones_f[:])
            nc.sync.dma_start(bv_row[:], bv[:])
            for pc in range(4):
                nc.sync.dma_start(bias_qk[:, pc : pc + 1], bq[pc * P : (pc + 1) * P, :])
                nc.sync.dma_start(
                    bias_qk[:, 4 + pc : 5 + pc], bk[pc * P : (pc + 1) * P, :]
                )

            # ---- Stage A: projections (x streamed in 512-token slices) ----
            for xdram, wdram, out_tiles, bcol in (
                (xq, wq, qt_t, 0),
                (xk, wk, kt_t, 4),
            ):
                wt = [
                    xw.tile([P, PJ], FP32R, tag="wt", name="wt") for _ in range(8)
                ]
                for dc in range(8):
                    nc.sync.dma_start(wt[dc][:], wdram[dc * P : (dc + 1) * P, :])
                for tk in range(4):
                    xt = [
                        xw.tile([P, 512], FP32R, tag="xt", name="xt")
                        for _ in range(8)
                    ]
                    for dc in range(8):
                        nc.sync.dma_start(
                            xt[dc][:],
                            xdram[dc * P : (dc + 1) * P, tk * 512 : (tk + 1) * 512],
                        )
                    for pc in range(4):
                        ps = psum_mm.tile([P, 512], FP32, tag="mm")
                        for dc in range(8):
                            nc.tensor.matmul(
                                ps[:],
                                wt[dc][:, pc * P : (pc + 1) * P],
                                xt[dc][:],
                                start=(dc == 0),
                                stop=(dc == 7),
                            )
                        nc.vector.tensor_scalar(
                            out_tiles[pc][:, tk * 512 : (tk + 1) * 512],
                            ps[:],
                            bias_qk[:, bcol + pc : bcol + pc + 1],
                            None,
                            ADD,
                        )
            # V: (token partition, (head, hd) free) + ones column.
            # Emitted as a generator so its PE-dense matmuls interleave with
            # unit 0's ACT-paced B2 stream (context mm kc needs v_t[kc]).
            def v_gen():
                wt = [
                    xw.tile([P, PJ], FP32R, tag="wt", name="wt") for _ in range(8)
                ]
                for dc in range(8):
                    nc.sync.dma_start(wt[dc][:], wv[dc * P : (dc + 1) * P, :])
                for tk in range(4):
                    xt = [
                        xw.tile([P, 512], FP32R, tag="xt", name="xt")
                        for _ in range(8)
                    ]
                    for dc in range(8):
                        nc.sync.dma_start(
                            xt[dc][:],
                            xv[dc * P : (dc + 1) * P, tk * 512 : (tk + 1) * 512],
                        )
                    for sub in range(4):
                        t16 = tk * 4 + sub
                        ps = psum_mm.tile([P, 512], FP32, tag="mm")
                        for dc in range(8):
                            nc.tensor.matmul(
                                ps[:],
                                xt[dc][:, sub * P : (sub + 1) * P],
                                wt[dc][:],
                                start=(dc == 0),
                                stop=False,
                            )
                        # + bv broadcast along tokens (rank-1)
                        nc.tensor.matmul(
                            ps[:], ones_t[0:1, :], bv_row[:], start=False, stop=True
                        )
                        nc.vector.tensor_copy(
                            v_t[t16][:, :, 0:HD],
                            ps[:].rearrange("p (h d) -> p h d", h=HG),
                        )
                        nc.vector.tensor_copy(v_t[t16][:, :, HD], ones_t[:, 0:HG])
                        yield

            # ---- Stage B: attention, per (head, 1024-query block) ----
            # Software-pipelined so the PE stream never dead-waits on ACT:
            # scoresT runs LAG iterations ahead of the context matmuls.
            LAG = 2

            def b1_scores(qt_h, kt_h, q0, qc):
                """Emit the 4 score matmuls for one 128-query chunk."""
                tiles = []
                for half in range(2):
                    ps_s = psum_mm.tile([P, 1024], FP32, tag="mm", name="ps_s")
                    for j in range(2):
                        k0 = half * 1024 + j * 512
                        nc.tensor.matmul(
                            ps_s[:, j * 512 : (j + 1) * 512],
                            qt_h[:, q0 + qc * P : q0 + (qc + 1) * P],
                            kt_h[:, k0 : k0 + 512],
                            start=True,
                            stop=True,
                        )
                    tiles.append(ps_s)
                return tiles

            def b1_expdma(h, q0, qc, ps_tiles, bcol):
                qg = q0 + qc * P
                for half in range(2):
                    at = work.tile([P, 1024], FP32, tag="attn", bufs=3, name="at")
                    nc.scalar.activation(
                        at[:], ps_tiles[half][:], Exp, scale=0.125, bias=bcol
                    )
                    nc.sync.dma_start(
                        attn_o[h, qg : qg + P, half * 1024 : half * 1024 + 1024],
                        at[:],
                    )

            def b2_unit(h, qb, result):
                """B2 for one unit as a generator: yields after each kc step
                (16+LAG yields), then one final yield before the fixup.
                Stores lnrec_cols into result dict for B1."""
                ti, po = h // 2, (h % 2) * HD
                qt_h = qt_t[ti][po : po + HD, :]
                kt_h = kt_t[ti][po : po + HD, :]
                ct_tile, cpo = ct_t[ti], po
                q0 = qb * 1024
                pc_ctx = psum_ctx.tile([HD + 1, 1024], FP32, tag="ctx")
                ets = {}
                for step in range(16 + LAG):
                    if step < 16:
                        kc = step
                        ps_t = psum_mm.tile([P, 1024], FP32, tag="mm", name="ps_t")
                        for j in range(2):
                            nc.tensor.matmul(
                                ps_t[:, j * 512 : (j + 1) * 512],
                                kt_h[:, kc * P : (kc + 1) * P],
                                qt_h[:, q0 + j * 512 : q0 + (j + 1) * 512],
                                start=True,
                                stop=True,
                            )
                        et = work.tile([P, 1024], FP32R, tag="expT", bufs=3,
                                       name="et")
                        nc.scalar.activation(et[:], ps_t[:], Exp, scale=0.125)
                        ets[kc] = et
                    if step >= LAG:
                        kc = step - LAG
                        et = ets.pop(kc)
                        for j in range(2):
                            nc.tensor.matmul(
                                pc_ctx[:, j * 512 : (j + 1) * 512],
                                v_t[kc][:, h, :],
                                et[:, j * 512 : (j + 1) * 512],
                                start=(kc == 0),
                                stop=(kc == 15),
                                skip_group_check=True,
                            )
                    yield
                yield  # caller may emit PE filler (B1 scores) here
                # Normalization fixup. Critical chain to B1's exp is only
                # sums->cols->recip->ln; everything else (context scale)
                # runs off PE's critical path (POOL broadcast + DVE).
                sums_sb = small.tile([1, 1024], FP32R, tag="sums", bufs=2)
                nc.vector.tensor_copy(sums_sb[:], pc_ctx[HD : HD + 1, :])
                sums_cols = psum_mm.tile([P, 16], FP32, tag="mm", name="scols")
                for qc in range(8):
                    nc.tensor.matmul(
                        sums_cols[:, 2 * qc : 2 * qc + 2],
                        sums_sb[0:1, qc * P : (qc + 1) * P],
                        ones_t[0:1, 0:2],
                        start=True,
                        stop=True,
                    )
                recip_cols = small.tile([P, 16], FP32, tag="rcols", bufs=2)
                nc.vector.reciprocal(recip_cols[:], sums_cols[:])
                lnrec_cols = small.tile([P, 16], FP32, tag="lncols", bufs=2)
                nc.scalar.activation(lnrec_cols[:], recip_cols[:], Ln)
                # context scale: recip back to row form, broadcast on POOL
                recip_row = small.tile([1, 1024], FP32, tag="rrow", bufs=1)
                for qc in range(8):
                    rt_ps = psum_mm.tile([1, P], FP32, tag="mm", name="rt_ps")
                    nc.tensor.transpose(
                        rt_ps[:], recip_cols[:, 2 * qc : 2 * qc + 1], ident_t[:]
                    )
                    nc.vector.tensor_copy(
                        recip_row[0:1, qc * P : (qc + 1) * P], rt_ps[:]
                    )
                bc_sb = work.tile([HD, 1024], FP32, tag="attn", bufs=3)
                nc.gpsimd.partition_broadcast(bc_sb[:], recip_row[:])
                nc.vector.tensor_tensor(
                    ct_tile[cpo : cpo + HD, q0 : q0 + 1024],
                    pc_ctx[0:HD, :],
                    bc_sb[:],
                    MULT,
                )
                result["lnrec"] = lnrec_cols

            def drive(gen, n):
                for _ in range(n):
                    if next(gen, "done") == "done":
                        return True
                return False

            units = [(h, qb) for h in range(HG) for qb in range(2)]
            results = [dict() for _ in units]
            gens = [b2_unit(h, qb, r) for (h, qb), r in zip(units, results)]
            # prologue: V projection interleaved with unit 0's B2 (V tiles
            # must stay ahead of the context matmuls consuming them)
            vg = v_gen()
            drive(vg, 8)
            for _ in range(8):
                drive(gens[0], 1)
                drive(vg, 1)
            drive(vg, 99)
            drive(gens[0], 9)

            for i, (h, qb) in enumerate(units):
                q0 = qb * 1024
                ti, po = h // 2, (h % 2) * HD
                qt_h = qt_t[ti][po : po + HD, :]
                kt_h = kt_t[ti][po : po + HD, :]
                # B1 qc=0 scores cover the fixup's recip/ln latency
                prev = b1_scores(qt_h, kt_h, q0, 0)
                drive(gens[i], 99)  # finish fixup of this unit
                lnrec_cols = results[i]["lnrec"]
                nxt_gen = gens[i + 1] if i + 1 < len(units) else None
                for qc in range(8):
                    nxt = b1_scores(qt_h, kt_h, q0, qc + 1) if qc < 7 else None
                    b1_expdma(h, q0, qc, prev, lnrec_cols[:, 2 * qc : 2 * qc + 1])
                    if nxt_gen is not None:
                        drive(nxt_gen, 2)  # interleave next unit's B2
                    prev = nxt
                # leave nxt_gen one step short of its fixup; the next loop
                # iteration emits its qc=0 scores first, then finishes it

            # ---- Stage C: output projection (partial) ----
            wot = [xw.tile([P, 512], FP32R, tag="wt", name="wot") for _ in range(8)]
            for pc in range(4):
                for oc in range(2):
                    nc.sync.dma_start(
                        wot[pc * 2 + oc][:],
                        wo[pc * P : (pc + 1) * P, oc * 512 : (oc + 1) * 512],
                    )
            for tk in range(16):
                for oc in range(2):
                    ps_o = psum_mm.tile([P, 512], FP32, tag="mm")
                    for pc in range(4):
                        nc.tensor.matmul(
                            ps_o[:],
                            ct_t[pc][:, tk * P : (tk + 1) * P],
                            wot[pc * 2 + oc][:],
                            start=(pc == 0),
                            stop=(pc == 3),
                        )
                    os_ = work.tile([P, 512], FP32, tag="attn", bufs=3, name="os_")
                    nc.scalar.copy(os_[:], ps_o[:])
                    nc.sync.dma_start(
                        out_p[tk * P : (tk + 1) * P, oc * 512 : (oc + 1) * 512], os_[:]
                    )

    nc.compile()
    return nc


_NC_CACHE = None


def _get_nc():
    global _NC_CACHE
    if _NC_CACHE is None:
        _NC_CACHE = build_nc()
    return _NC_CACHE


def _prep_in_maps(inputs):
    query = np.asarray(inputs["query"], np.float32)
    key = np.asarray(inputs["key"], np.float32)
    value = np.asarray(inputs["value"], np.float32)
    Wq = np.asarray(inputs["Wq"], np.float32)
    Wk = np.asarray(inputs["Wk"], np.float32)
    Wv = np.asarray(inputs["Wv"], np.float32)
    Wo = np.asarray(inputs["Wo"], np.float32)
    bq = np.asarray(inputs["bq"], np.float32)
    bk = np.asarray(inputs["bk"], np.float32)
    bv = np.asarray(inputs["bv"], np.float32)

    # per-batch and per-headgroup pieces are shared between cores
    xb = [
        {
            "xq": round_fp32r(query[b].T),
            "xk": round_fp32r(key[b].T),
            "xv": round_fp32r(value[b].T),
        }
        for b in range(B)
    ]
    ident = np.eye(P, dtype=np.float32)
    wg = []
    for g in range(2):
        sl = slice(g * PJ, (g + 1) * PJ)
        wg.append(
            {
                "wq": round_fp32r(np.ascontiguousarray(Wq[sl, :].T)),
                "wk": round_fp32r(np.ascontiguousarray(Wk[sl, :].T)),
                "wv": round_fp32r(np.ascontiguousarray(Wv[sl, :].T)),
                "wo": round_fp32r(np.ascontiguousarray(Wo[:, sl].T)),
                "bq": np.ascontiguousarray(bq[sl]).reshape(PJ, 1),
                "bk": np.ascontiguousarray(bk[sl]).reshape(PJ, 1),
                "bv": round_fp32r(bv[sl]).reshape(1, PJ),
                "ident": ident,
            }
        )
    return [{**xb[c // 2], **wg[c % 2]} for c in range(NCORES)]


def kernel(query, key, value, Wq, bq, Wk, bk, Wv, bv, Wo, bo, **_kw):
    bo = np.asarray(bo, np.float32)
    nc = _get_nc()
    in_maps = _prep_in_maps(
        dict(
            query=query, key=key, value=value, Wq=Wq, Wk=Wk, Wv=Wv, Wo=Wo,
            bq=bq, bk=bk, bv=bv,
        )
    )
    res = run_bass_kernel_spmd(nc, in_maps, core_ids=list(range(NCORES)))

    attn = np.empty((B, NH, S, S), np.float32)
    out = np.empty((B, S, D), np.float32)
    for b in range(B):
        r0, r1 = res.results[2 * b], res.results[2 * b + 1]
        attn[b, 0:HG] = r0["attn_o"]
        attn[b, HG:NH] = r1["attn_o"]
        out[b] = r0["out_p"] + r1["out_p"] + bo[None, :]
    return out, attn


# revision 28
# speedup vs baseline: 1.0051x; 1.0051x over previous
"""Multi-head attention (B=4, S=2048, D=1024, H=16, hd=64) on 8 trn2 cores.

Sharding: core c -> batch b = c//2, head-group g = c%2 (8 heads = 512 proj dims).
Each core computes QKV projections for its batch with head-sliced weights,
full attention for its 8 heads (attn weights are part of the output), and a
partial output projection. Host sums the two partial outputs per batch and
concatenates attention slices.

All matmuls run in float32r (TF32-like e8m11, full PE rate). Softmax
normalization is folded into the ACT exp as a per-partition -ln(rowsum) bias;
rowsums come from a ones-row appended to V in the transposed-orientation
context matmul.
"""

import sys

if "/opt/trn_rl_repo" not in sys.path:
    sys.path.insert(0, "/opt/trn_rl_repo")

import numpy as np

import concourse.bass as bass
import concourse.mybir as mybir
import concourse.tile as tile
from concourse import bacc
from concourse.bass_utils import run_bass_kernel_spmd

FP32 = mybir.dt.float32
FP32R = mybir.dt.float32r

B, S, D = 4, 2048, 1024
NH, HD = 16, 64
HG = 8  # heads per core
PJ = HG * HD  # 512 proj dims per core
P = 128
NCORES = 8

Exp = mybir.ActivationFunctionType.Exp
Ln = mybir.ActivationFunctionType.Ln
ADD = mybir.AluOpType.add
MULT = mybir.AluOpType.mult


def round_fp32r(x: np.ndarray) -> np.ndarray:
    """Round fp32 to fp32r (e8m11, RNE) — what the HW datapath expects.

    uint32 arithmetic wraps only for NaN-pattern inputs (not produced here).
    """
    u = np.ascontiguousarray(x).view(np.uint32)
    r = (u + np.uint32(0x7FF) + ((u >> np.uint32(12)) & np.uint32(1))) & np.uint32(
        0xFFFFF000
    )
    return r.view(np.float32)


class _Bacc(bacc.Bacc):
    """Bacc with ACT-table thrash removed: every table load targets
    natural_log_exp_and_others (contains exp, ln, copy — all we use), and
    redundant same-set loads are dropped."""

    def insert_act_table_loads(self):
        super().insert_act_table_loads()
        from concourse.hw_specs import get_activation_tables

        names = list(get_activation_tables(self.m.arch).keys())
        if "natural_log_exp_and_others" not in names:
            return
        target = names.index("natural_log_exp_and_others")
        for blk in self.main_func.blocks:
            keep = []
            seen = False
            for ins in blk.instructions:
                if isinstance(ins, mybir.InstLoadActFuncSet):
                    if seen:
                        continue
                    ins.act_func_set_id = target
                    seen = True
                keep.append(ins)
            if len(keep) != len(blk.instructions):
                del blk.instructions[:]
                for ins in keep:
                    blk.instructions.append(ins)


def build_nc():
    nc = _Bacc(None, target_bir_lowering=False, debug=False)

    xq = nc.dram_tensor("xq", [D, S], FP32R, kind="ExternalInput")
    xk = nc.dram_tensor("xk", [D, S], FP32R, kind="ExternalInput")
    xv = nc.dram_tensor("xv", [D, S], FP32R, kind="ExternalInput")
    wq = nc.dram_tensor("wq", [D, PJ], FP32R, kind="ExternalInput")
    wk = nc.dram_tensor("wk", [D, PJ], FP32R, kind="ExternalInput")
    wv = nc.dram_tensor("wv", [D, PJ], FP32R, kind="ExternalInput")
    wo = nc.dram_tensor("wo", [PJ, D], FP32R, kind="ExternalInput")
    bq = nc.dram_tensor("bq", [PJ, 1], FP32, kind="ExternalInput")
    bk = nc.dram_tensor("bk", [PJ, 1], FP32, kind="ExternalInput")
    bv = nc.dram_tensor("bv", [1, PJ], FP32R, kind="ExternalInput")
    ident = nc.dram_tensor("ident", [P, P], FP32, kind="ExternalInput")

    attn_o = nc.dram_tensor("attn_o", [HG, S, S], FP32, kind="ExternalOutput")
    out_p = nc.dram_tensor("out_p", [S, D], FP32, kind="ExternalOutput")

    with tile.TileContext(nc) as tc:
        with (
            tc.tile_pool(name="persist", bufs=1) as persist,
            tc.tile_pool(name="xw", bufs=8) as xw,
            tc.tile_pool(name="work", bufs=2) as work,
            tc.tile_pool(name="small", bufs=2) as small,
            tc.tile_pool(name="psum_mm", bufs=3, space="PSUM") as psum_mm,
            tc.tile_pool(name="psum_ctx", bufs=1, space="PSUM") as psum_ctx,
        ):
            # ---- persistent SBUF tensors ----
            qt_t = [
                persist.tile([P, S], FP32R, tag=f"qt{i}", name=f"qt{i}")
                for i in range(4)
            ]
            kt_t = [
                persist.tile([P, S], FP32R, tag=f"kt{i}", name=f"kt{i}")
                for i in range(4)
            ]
            # V with a ones column per head: (128, 8, 65)
            v_t = [
                persist.tile([P, HG, HD + 1], FP32R, tag=f"v{i}", name=f"v{i}")
                for i in range(16)
            ]
            ct_t = [
                persist.tile([P, S], FP32R, tag=f"ct{i}", name=f"ct{i}")
                for i in range(4)
            ]
            bias_qk = persist.tile([P, 8], FP32, tag="bias_qk")  # bq 0:4, bk 4:8
            bv_row = persist.tile([1, PJ], FP32R, tag="bv_row")
            # ones (fp32r) usable from base partitions 0 and 64
            ones_t = persist.tile([P, P], FP32R, tag="ones_t")

            ident_t = persist.tile([P, P], FP32, tag="ident_t")
            nc.sync.dma_start(ident_t[:], ident[:])
            ones_f = small.tile([P, P], FP32, tag="ones_f", bufs=1)
            nc.vector.memset(ones_f[:], 1.0)
            nc.vector.tensor_copy(ones_t[:], ones_f[:])
            nc.sync.dma_start(bv_row[:], bv[:])
            for pc in range(4):
                nc.sync.dma_start(bias_qk[:, pc : pc + 1], bq[pc * P : (pc + 1) * P, :])
                nc.sync.dma_start(
                    bias_qk[:, 4 + pc : 5 + pc], bk[pc * P : (pc + 1) * P, :]
                )

            # ---- Stage A: projections (x streamed in 512-token slices) ----
            for xdram, wdram, out_tiles, bcol in (
                (xq, wq, qt_t, 0),
                (xk, wk, kt_t, 4),
            ):
                wt = [
                    xw.tile([P, PJ], FP32R, tag="wt", name="wt") for _ in range(8)
                ]
                for dc in range(8):
                    nc.sync.dma_start(wt[dc][:], wdram[dc * P : (dc + 1) * P, :])
                for tk in range(4):
                    xt = [
                        xw.tile([P, 512], FP32R, tag="xt", name="xt")
                        for _ in range(8)
                    ]
                    for dc in range(8):
                        nc.sync.dma_start(
                            xt[dc][:],
                            xdram[dc * P : (dc + 1) * P, tk * 512 : (tk + 1) * 512],
                        )
                    for pc in range(4):
                        ps = psum_mm.tile([P, 512], FP32, tag="mm")
                        for dc in range(8):
                            nc.tensor.matmul(
                                ps[:],
                                wt[dc][:, pc * P : (pc + 1) * P],
                                xt[dc][:],
                                start=(dc == 0),
                                stop=(dc == 7),
                            )
                        nc.vector.tensor_scalar(
                            out_tiles[pc][:, tk * 512 : (tk + 1) * 512],
                            ps[:],
                            bias_qk[:, bcol + pc : bcol + pc + 1],
                            None,
                            ADD,
                        )
            # V: (token partition, (head, hd) free) + ones column.
            # Emitted as a generator so its PE-dense matmuls interleave with
            # unit 0's ACT-paced B2 stream (context mm kc needs v_t[kc]).
            def v_gen():
                wt = [
                    xw.tile([P, PJ], FP32R, tag="wt", name="wt") for _ in range(8)
                ]
                for dc in range(8):
                    nc.sync.dma_start(wt[dc][:], wv[dc * P : (dc + 1) * P, :])
                for tk in range(4):
                    xt = [
                        xw.tile([P, 512], FP32R, tag="xt", name="xt")
                        for _ in range(8)
                    ]
                    for dc in range(8):
                        nc.sync.dma_start(
                            xt[dc][:],
                            xv[dc * P : (dc + 1) * P, tk * 512 : (tk + 1) * 512],
                        )
                    for sub in range(4):
                        t16 = tk * 4 + sub
                        ps = psum_mm.tile([P, 512], FP32, tag="mm")
                        for dc in range(8):
                            nc.tensor.matmul(
                                ps[:],
                                xt[dc][:, sub * P : (sub + 1) * P],
                                wt[dc][:],
                                start=(dc == 0),
                                stop=False,
                            )
                        # + bv broadcast along tokens (rank-1)
                        nc.tensor.matmul(
                            ps[:], ones_t[0:1, :], bv_row[:], start=False, stop=True
                        )
                        nc.vector.tensor_copy(
                            v_t[t16][:, :, 0:HD],
                            ps[:].rearrange("p (h d) -> p h d", h=HG),
                        )
                        nc.vector.tensor_copy(v_t[t16][:, :, HD], ones_t[:, 0:HG])
                        yield

            # ---- Stage B: attention, per (head, 1024-query block) ----
            # Software-pipelined so the PE stream never dead-waits on ACT:
            # scoresT runs LAG iterations ahead of the context matmuls.
            LAG = 2

            def b1_scores(qt_h, kt_h, q0, qc):
                """Emit the 4 score matmuls for one 128-query chunk."""
                tiles = []
                for half in range(2):
                    ps_s = psum_mm.tile([P, 1024], FP32, tag="mm", name="ps_s")
                    for j in range(2):
                        k0 = half * 1024 + j * 512
                        nc.tensor.matmul(
                            ps_s[:, j * 512 : (j + 1) * 512],
                            qt_h[:, q0 + qc * P : q0 + (qc + 1) * P],
                            kt_h[:, k0 : k0 + 512],
                            start=True,
                            stop=True,
                        )
                    tiles.append(ps_s)
                return tiles

            def b1_expdma(h, q0, qc, ps_tiles, bcol):
                qg = q0 + qc * P
                for half in range(2):
                    at = work.tile([P, 1024], FP32, tag="attn", bufs=3, name="at")
                    nc.scalar.activation(
                        at[:], ps_tiles[half][:], Exp, scale=0.125, bias=bcol
                    )
                    nc.sync.dma_start(
                        attn_o[h, qg : qg + P, half * 1024 : half * 1024 + 1024],
                        at[:],
                    )

            def b2_unit(h, qb, result):
                """B2 for one unit as a generator: yields after each kc step
                (16+LAG yields), then one final yield before the fixup.
                Stores lnrec_cols into result dict for B1."""
                ti, po = h // 2, (h % 2) * HD
                qt_h = qt_t[ti][po : po + HD, :]
                kt_h = kt_t[ti][po : po + HD, :]
                ct_tile, cpo = ct_t[ti], po
                q0 = qb * 1024
                pc_ctx = psum_ctx.tile([HD + 1, 1024], FP32, tag="ctx")
                ets = {}
                for step in range(16 + LAG):
                    if step < 16:
                        kc = step
                        ps_t = psum_mm.tile([P, 1024], FP32, tag="mm", name="ps_t")
                        for j in range(2):
                            nc.tensor.matmul(
                                ps_t[:, j * 512 : (j + 1) * 512],
                                kt_h[:, kc * P : (kc + 1) * P],
                                qt_h[:, q0 + j * 512 : q0 + (j + 1) * 512],
                                start=True,
                                stop=True,
                            )
                        et = work.tile([P, 1024], FP32R, tag="expT", bufs=3,
                                       name="et")
                        nc.scalar.activation(et[:], ps_t[:], Exp, scale=0.125)
                        ets[kc] = et
                    if step >= LAG:
                        kc = step - LAG
                        et = ets.pop(kc)
                        for j in range(2):
                            nc.tensor.matmul(
                                pc_ctx[:, j * 512 : (j + 1) * 512],
                                v_t[kc][:, h, :],
                                et[:, j * 512 : (j + 1) * 512],
                                start=(kc == 0),
                                stop=(kc == 15),
                                skip_group_check=True,
                            )
                    yield
                yield  # caller may emit PE filler (B1 scores) here
                # Normalization fixup. Critical chain to B1's exp is only
                # sums->cols->recip->ln; everything else (context scale)
                # runs off PE's critical path (POOL broadcast + DVE).
                sums_sb = small.tile([1, 1024], FP32R, tag="sums", bufs=2)
                nc.vector.tensor_copy(sums_sb[:], pc_ctx[HD : HD + 1, :])
                sums_cols = psum_mm.tile([P, 16], FP32, tag="mm", name="scols")
                for qc in range(8):
                    nc.tensor.matmul(
                        sums_cols[:, 2 * qc : 2 * qc + 2],
                        sums_sb[0:1, qc * P : (qc + 1) * P],
                        ones_t[0:1, 0:2],
                        start=True,
                        stop=True,
                    )
                recip_cols = small.tile([P, 16], FP32, tag="rcols", bufs=2)
                nc.vector.reciprocal(recip_cols[:], sums_cols[:])
                lnrec_cols = small.tile([P, 16], FP32, tag="lncols", bufs=2)
                nc.scalar.activation(lnrec_cols[:], recip_cols[:], Ln)
                # context scale: recip back to row form, broadcast on POOL
                recip_row = small.tile([1, 1024], FP32, tag="rrow", bufs=1)
                for qc in range(8):
                    rt_ps = psum_mm.tile([1, P], FP32, tag="mm", name="rt_ps")
                    nc.tensor.transpose(
                        rt_ps[:], recip_cols[:, 2 * qc : 2 * qc + 1], ident_t[:]
                    )
                    nc.vector.tensor_copy(
                        recip_row[0:1, qc * P : (qc + 1) * P], rt_ps[:]
                    )
                bc_sb = work.tile([HD, 1024], FP32, tag="attn", bufs=3)
                nc.gpsimd.partition_broadcast(bc_sb[:], recip_row[:])
                nc.vector.tensor_tensor(
                    ct_tile[cpo : cpo + HD, q0 : q0 + 1024],
                    pc_ctx[0:HD, :],
                    bc_sb[:],
                    MULT,
                )
                result["lnrec"] = lnrec_cols

            def drive(gen, n):
                for _ in range(n):
                    if next(gen, "done") == "done":
                        return True
                return False

            units = [(h, qb) for h in range(HG) for qb in range(2)]
            results = [dict() for _ in units]
            gens = [b2_unit(h, qb, r) for (h, qb), r in zip(units, results)]
            # prologue: V projection interleaved with unit 0's B2 (V tiles
            # must stay ahead of the context matmuls consuming them)
            vg = v_gen()
            drive(vg, 8)
            for _ in range(8):
                drive(gens[0], 1)
                drive(vg, 1)
            drive(vg, 99)
            drive(gens[0], 9)

            for i, (h, qb) in enumerate(units):
                q0 = qb * 1024
                ti, po = h // 2, (h % 2) * HD
                qt_h = qt_t[ti][po : po + HD, :]
                kt_h = kt_t[ti][po : po + HD, :]
                # B1 qc=0 scores cover the fixup's recip/ln latency
                prev = b1_scores(qt_h, kt_h, q0, 0)
                drive(gens[i], 99)  # finish fixup of this unit
                lnrec_cols = results[i]["lnrec"]
                nxt_gen = gens[i + 1] if i + 1 < len(units) else None
                for qc in range(8):
                    nxt = b1_scores(qt_h, kt_h, q0, qc + 1) if qc < 7 else None
                    b1_expdma(h, q0, qc, prev, lnrec_cols[:, 2 * qc : 2 * qc + 1])
                    if nxt_gen is not None:
                        drive(nxt_gen, 2)  # interleave next unit's B2
                    prev = nxt
                # leave nxt_gen one step short of its fixup; the next loop
                # iteration emits its qc=0 scores first, then finishes it

            # ---- Stage C: output projection (partial) ----
            wot = [xw.tile([P, 512], FP32R, tag="wt", name="wot") for _ in range(8)]
            for pc in range(4):
                for oc in range(2):
                    nc.sync.dma_start(
                        wot[pc * 2 + oc][:],
                        wo[pc * P : (pc + 1) * P, oc * 512 : (oc + 1) * 512],
                    )
            for tk in range(16):
                for oc in range(2):
                    ps_o = psum_mm.tile([P, 512], FP32, tag="mm")
                    for pc in range(4):
                        nc.tensor.matmul(
                            ps_o[:],
                            ct_t[pc][:, tk * P : (tk + 1) * P],
                            wot[pc * 2 + oc][:],
                            start=(pc == 0),
                            stop=(pc == 3),
                        )
                    os_ = work.tile([P, 512], FP32, tag="attn", bufs=3, name="os_")
                    nc.scalar.copy(os_[:], ps_o[:])
                    nc.sync.dma_start(
                        out_p[tk * P : (tk + 1) * P, oc * 512 : (oc + 1) * 512], os_[:]
                    )

    nc.compile()
    return nc


_NC_CACHE = None


def _get_nc():
    global _NC_CACHE
    if _NC_CACHE is None:
        _NC_CACHE = build_nc()
    return _NC_CACHE


def _prep_in_maps(inputs):
    query = np.asarray(inputs["query"], np.float32)
    key = np.asarray(inputs["key"], np.float32)
    value = np.asarray(inputs["value"], np.float32)
    Wq = np.asarray(inputs["Wq"], np.float32)
    Wk = np.asarray(inputs["Wk"], np.float32)
    Wv = np.asarray(inputs["Wv"], np.float32)
    Wo = np.asarray(inputs["Wo"], np.float32)
    bq = np.asarray(inputs["bq"], np.float32)
    bk = np.asarray(inputs["bk"], np.float32)
    bv = np.asarray(inputs["bv"], np.float32)

    # per-batch and per-headgroup pieces are shared between cores
    xb = [
        {
            "xq": round_fp32r(query[b].T),
            "xk": round_fp32r(key[b].T),
            "xv": round_fp32r(value[b].T),
        }
        for b in range(B)
    ]
    ident = np.eye(P, dtype=np.float32)
    wg = []
    for g in range(2):
        sl = slice(g * PJ, (g + 1) * PJ)
        wg.append(
            {
                "wq": round_fp32r(np.ascontiguousarray(Wq[sl, :].T)),
                "wk": round_fp32r(np.ascontiguousarray(Wk[sl, :].T)),
                "wv": round_fp32r(np.ascontiguousarray(Wv[sl, :].T)),
                "wo": round_fp32r(np.ascontiguousarray(Wo[:, sl].T)),
                "bq": np.ascontiguousarray(bq[sl]).reshape(PJ, 1),
                "bk": np.ascontiguousarray(bk[sl]).reshape(PJ, 1),
                "bv": round_fp32r(bv[sl]).reshape(1, PJ),
                "ident": ident,
            }
        )
    return [{**xb[c // 2], **wg[c % 2]} for c in range(NCORES)]


def kernel(query, key, value, Wq, bq, Wk, bk, Wv, bv, Wo, bo, **_kw):
    bo = np.asarray(bo, np.float32)
    nc = _get_nc()
    in_maps = _prep_in_maps(
        dict(
            query=query, key=key, value=value, Wq=Wq, Wk=Wk, Wv=Wv, Wo=Wo,
            bq=bq, bk=bk, bv=bv,
        )
    )
    res = run_bass_kernel_spmd(nc, in_maps, core_ids=list(range(NCORES)))

    attn = np.empty((B, NH, S, S), np.float32)
    out = np.empty((B, S, D), np.float32)
    for b in range(B):
        r0, r1 = res.results[2 * b], res.results[2 * b + 1]
        attn[b, 0:HG] = r0["attn_o"]
        attn[b, HG:NH] = r1["attn_o"]
        out[b] = r0["out_p"] + r1["out_p"] + bo[None, :]
    return out, attn


# revision 29
# speedup vs baseline: 1.0236x; 1.0184x over previous
"""Multi-head attention (B=4, S=2048, D=1024, H=16, hd=64) on 8 trn2 cores.

Sharding: core c -> batch b = c//2, head-group g = c%2 (8 heads = 512 proj dims).
Each core computes QKV projections for its batch with head-sliced weights,
full attention for its 8 heads (attn weights are part of the output), and a
partial output projection. Host sums the two partial outputs per batch and
concatenates attention slices.

All matmuls run in float32r (TF32-like e8m11, full PE rate). Softmax
normalization is folded into the ACT exp as a per-partition -ln(rowsum) bias;
rowsums come from a ones-row appended to V in the transposed-orientation
context matmul.
"""

import sys

if "/opt/trn_rl_repo" not in sys.path:
    sys.path.insert(0, "/opt/trn_rl_repo")

import numpy as np

import concourse.bass as bass
import concourse.mybir as mybir
import concourse.tile as tile
from concourse import bacc
from concourse.bass_utils import run_bass_kernel_spmd

FP32 = mybir.dt.float32
FP32R = mybir.dt.float32r

B, S, D = 4, 2048, 1024
NH, HD = 16, 64
HG = 8  # heads per core
PJ = HG * HD  # 512 proj dims per core
P = 128
NCORES = 8

Exp = mybir.ActivationFunctionType.Exp
Ln = mybir.ActivationFunctionType.Ln
ADD = mybir.AluOpType.add
MULT = mybir.AluOpType.mult


def round_fp32r(x: np.ndarray) -> np.ndarray:
    """Round fp32 to fp32r (e8m11, RNE) — what the HW datapath expects.

    uint32 arithmetic wraps only for NaN-pattern inputs (not produced here).
    """
    u = np.ascontiguousarray(x).view(np.uint32)
    r = (u + np.uint32(0x7FF) + ((u >> np.uint32(12)) & np.uint32(1))) & np.uint32(
        0xFFFFF000
    )
    return r.view(np.float32)


class _Bacc(bacc.Bacc):
    """Bacc with ACT-table thrash removed: every table load targets
    natural_log_exp_and_others (contains exp, ln, copy — all we use), and
    redundant same-set loads are dropped."""

    def insert_act_table_loads(self):
        super().insert_act_table_loads()
        from concourse.hw_specs import get_activation_tables

        names = list(get_activation_tables(self.m.arch).keys())
        if "natural_log_exp_and_others" not in names:
            return
        target = names.index("natural_log_exp_and_others")
        for blk in self.main_func.blocks:
            keep = []
            seen = False
            for ins in blk.instructions:
                if isinstance(ins, mybir.InstLoadActFuncSet):
                    if seen:
                        continue
                    ins.act_func_set_id = target
                    seen = True
                keep.append(ins)
            if len(keep) != len(blk.instructions):
                del blk.instructions[:]
                for ins in keep:
                    blk.instructions.append(ins)


def build_nc():
    nc = _Bacc(None, target_bir_lowering=False, debug=False)

    xq = nc.dram_tensor("xq", [D, S], FP32R, kind="ExternalInput")
    xk = nc.dram_tensor("xk", [D, S], FP32R, kind="ExternalInput")
    xv = nc.dram_tensor("xv", [D, S], FP32R, kind="ExternalInput")
    wq = nc.dram_tensor("wq", [D, PJ], FP32R, kind="ExternalInput")
    wk = nc.dram_tensor("wk", [D, PJ], FP32R, kind="ExternalInput")
    wv = nc.dram_tensor("wv", [D, PJ], FP32R, kind="ExternalInput")
    wo = nc.dram_tensor("wo", [PJ, D], FP32R, kind="ExternalInput")
    bq = nc.dram_tensor("bq", [PJ, 1], FP32, kind="ExternalInput")
    bk = nc.dram_tensor("bk", [PJ, 1], FP32, kind="ExternalInput")
    bv = nc.dram_tensor("bv", [1, PJ], FP32R, kind="ExternalInput")
    ident = nc.dram_tensor("ident", [P, P], FP32, kind="ExternalInput")

    attn_o = nc.dram_tensor("attn_o", [HG, S, S], FP32, kind="ExternalOutput")
    out_p = nc.dram_tensor("out_p", [S, D], FP32, kind="ExternalOutput")

    with tile.TileContext(nc) as tc:
        with (
            tc.tile_pool(name="persist", bufs=1) as persist,
            tc.tile_pool(name="xw", bufs=8) as xw,
            tc.tile_pool(name="work", bufs=2) as work,
            tc.tile_pool(name="small", bufs=2) as small,
            tc.tile_pool(name="psum_mm", bufs=3, space="PSUM") as psum_mm,
            tc.tile_pool(name="psum_ctx", bufs=1, space="PSUM") as psum_ctx,
        ):
            # ---- persistent SBUF tensors ----
            qt_t = [
                persist.tile([P, S], FP32R, tag=f"qt{i}", name=f"qt{i}")
                for i in range(4)
            ]
            kt_t = [
                persist.tile([P, S], FP32R, tag=f"kt{i}", name=f"kt{i}")
                for i in range(4)
            ]
            # V with a ones column per head: (128, 8, 65)
            v_t = [
                persist.tile([P, HG, HD + 1], FP32R, tag=f"v{i}", name=f"v{i}")
                for i in range(16)
            ]
            ct_t = [
                persist.tile([P, S], FP32R, tag=f"ct{i}", name=f"ct{i}")
                for i in range(4)
            ]
            bias_qk = persist.tile([P, 8], FP32, tag="bias_qk")  # bq 0:4, bk 4:8
            bv_row = persist.tile([1, PJ], FP32R, tag="bv_row")
            # ones (fp32r) usable from base partitions 0 and 64
            ones_t = persist.tile([P, P], FP32R, tag="ones_t")

            ident_t = persist.tile([P, P], FP32, tag="ident_t")
            nc.sync.dma_start(ident_t[:], ident[:])
            ones_f = small.tile([P, P], FP32, tag="ones_f", bufs=1)
            nc.vector.memset(ones_f[:], 1.0)
            nc.vector.tensor_copy(ones_t[:], ones_f[:])
            nc.sync.dma_start(bv_row[:], bv[:])
            for pc in range(4):
                nc.sync.dma_start(bias_qk[:, pc : pc + 1], bq[pc * P : (pc + 1) * P, :])
                nc.sync.dma_start(
                    bias_qk[:, 4 + pc : 5 + pc], bk[pc * P : (pc + 1) * P, :]
                )

            # ---- Stage A: projections (x streamed in 512-token slices) ----
            for xdram, wdram, out_tiles, bcol in (
                (xq, wq, qt_t, 0),
                (xk, wk, kt_t, 4),
            ):
                wt = [
                    xw.tile([P, PJ], FP32R, tag="wt", name="wt") for _ in range(8)
                ]
                for dc in range(8):
                    nc.sync.dma_start(wt[dc][:], wdram[dc * P : (dc + 1) * P, :])
                for tk in range(8):
                    xt = [
                        xw.tile([P, 256], FP32R, tag="xt", name="xt")
                        for _ in range(8)
                    ]
                    for dc in range(8):
                        nc.sync.dma_start(
                            xt[dc][:],
                            xdram[dc * P : (dc + 1) * P, tk * 256 : (tk + 1) * 256],
                        )
                    for pc in range(4):
                        ps = psum_mm.tile([P, 256], FP32, tag="mm")
                        for dc in range(8):
                            nc.tensor.matmul(
                                ps[:],
                                wt[dc][:, pc * P : (pc + 1) * P],
                                xt[dc][:],
                                start=(dc == 0),
                                stop=(dc == 7),
                            )
                        nc.vector.tensor_scalar(
                            out_tiles[pc][:, tk * 256 : (tk + 1) * 256],
                            ps[:],
                            bias_qk[:, bcol + pc : bcol + pc + 1],
                            None,
                            ADD,
                        )
            # V: (token partition, (head, hd) free) + ones column.
            # Emitted as a generator so its PE-dense matmuls interleave with
            # unit 0's ACT-paced B2 stream (context mm kc needs v_t[kc]).
            def v_gen():
                wt = [
                    xw.tile([P, PJ], FP32R, tag="wt", name="wt") for _ in range(8)
                ]
                for dc in range(8):
                    nc.sync.dma_start(wt[dc][:], wv[dc * P : (dc + 1) * P, :])
                for tk in range(4):
                    xt = [
                        xw.tile([P, 512], FP32R, tag="xt", name="xt")
                        for _ in range(8)
                    ]
                    for dc in range(8):
                        nc.sync.dma_start(
                            xt[dc][:],
                            xv[dc * P : (dc + 1) * P, tk * 512 : (tk + 1) * 512],
                        )
                    for sub in range(4):
                        t16 = tk * 4 + sub
                        ps = psum_mm.tile([P, 512], FP32, tag="mm")
                        for dc in range(8):
                            nc.tensor.matmul(
                                ps[:],
                                xt[dc][:, sub * P : (sub + 1) * P],
                                wt[dc][:],
                                start=(dc == 0),
                                stop=False,
                            )
                        # + bv broadcast along tokens (rank-1)
                        nc.tensor.matmul(
                            ps[:], ones_t[0:1, :], bv_row[:], start=False, stop=True
                        )
                        nc.vector.tensor_copy(
                            v_t[t16][:, :, 0:HD],
                            ps[:].rearrange("p (h d) -> p h d", h=HG),
                        )
                        nc.vector.tensor_copy(v_t[t16][:, :, HD], ones_t[:, 0:HG])
                        yield

            # ---- Stage B: attention, per (head, 1024-query block) ----
            # Software-pipelined so the PE stream never dead-waits on ACT:
            # scoresT runs LAG iterations ahead of the context matmuls.
            LAG = 2

            def b1_scores(qt_h, kt_h, q0, qc):
                """Emit the 4 score matmuls for one 128-query chunk."""
                tiles = []
                for half in range(2):
                    ps_s = psum_mm.tile([P, 1024], FP32, tag="mm", name="ps_s")
                    for j in range(2):
                        k0 = half * 1024 + j * 512
                        nc.tensor.matmul(
                            ps_s[:, j * 512 : (j + 1) * 512],
                            qt_h[:, q0 + qc * P : q0 + (qc + 1) * P],
                            kt_h[:, k0 : k0 + 512],
                            start=True,
                            stop=True,
                        )
                    tiles.append(ps_s)
                return tiles

            def b1_expdma(h, q0, qc, ps_tiles, bcol):
                qg = q0 + qc * P
                for half in range(2):
                    at = work.tile([P, 1024], FP32, tag="attn", bufs=3, name="at")
                    nc.scalar.activation(
                        at[:], ps_tiles[half][:], Exp, scale=0.125, bias=bcol
                    )
                    nc.sync.dma_start(
                        attn_o[h, qg : qg + P, half * 1024 : half * 1024 + 1024],
                        at[:],
                    )

            def b2_unit(h, qb, result):
                """B2 for one unit as a generator: yields after each kc step
                (16+LAG yields), then one final yield before the fixup.
                Stores lnrec_cols into result dict for B1."""
                ti, po = h // 2, (h % 2) * HD
                qt_h = qt_t[ti][po : po + HD, :]
                kt_h = kt_t[ti][po : po + HD, :]
                ct_tile, cpo = ct_t[ti], po
                q0 = qb * 1024
                pc_ctx = psum_ctx.tile([HD + 1, 1024], FP32, tag="ctx")
                ets = {}
                for step in range(16 + LAG):
                    if step < 16:
                        kc = step
                        ps_t = psum_mm.tile([P, 1024], FP32, tag="mm", name="ps_t")
                        for j in range(2):
                            nc.tensor.matmul(
                                ps_t[:, j * 512 : (j + 1) * 512],
                                kt_h[:, kc * P : (kc + 1) * P],
                                qt_h[:, q0 + j * 512 : q0 + (j + 1) * 512],
                                start=True,
                                stop=True,
                            )
                        et = work.tile([P, 1024], FP32R, tag="expT", bufs=3,
                                       name="et")
                        nc.scalar.activation(et[:], ps_t[:], Exp, scale=0.125)
                        ets[kc] = et
                    if step >= LAG:
                        kc = step - LAG
                        et = ets.pop(kc)
                        for j in range(2):
                            nc.tensor.matmul(
                                pc_ctx[:, j * 512 : (j + 1) * 512],
                                v_t[kc][:, h, :],
                                et[:, j * 512 : (j + 1) * 512],
                                start=(kc == 0),
                                stop=(kc == 15),
                                skip_group_check=True,
                            )
                    yield
                yield  # caller may emit PE filler (B1 scores) here
                # Normalization fixup. Critical chain to B1's exp is only
                # sums->cols->recip->ln; everything else (context scale)
                # runs off PE's critical path (POOL broadcast + DVE).
                sums_sb = small.tile([1, 1024], FP32R, tag="sums", bufs=2)
                nc.vector.tensor_copy(sums_sb[:], pc_ctx[HD : HD + 1, :])
                sums_cols = psum_mm.tile([P, 16], FP32, tag="mm", name="scols")
                for qc in range(8):
                    nc.tensor.matmul(
                        sums_cols[:, 2 * qc : 2 * qc + 2],
                        sums_sb[0:1, qc * P : (qc + 1) * P],
                        ones_t[0:1, 0:2],
                        start=True,
                        stop=True,
                    )
                recip_cols = small.tile([P, 16], FP32, tag="rcols", bufs=2)
                nc.vector.reciprocal(recip_cols[:], sums_cols[:])
                lnrec_cols = small.tile([P, 16], FP32, tag="lncols", bufs=2)
                nc.scalar.activation(lnrec_cols[:], recip_cols[:], Ln)
                # context scale: recip back to row form, broadcast on POOL
                recip_row = small.tile([1, 1024], FP32, tag="rrow", bufs=1)
                for qc in range(8):
                    rt_ps = psum_mm.tile([1, P], FP32, tag="mm", name="rt_ps")
                    nc.tensor.transpose(
                        rt_ps[:], recip_cols[:, 2 * qc : 2 * qc + 1], ident_t[:]
                    )
                    nc.vector.tensor_copy(
                        recip_row[0:1, qc * P : (qc + 1) * P], rt_ps[:]
                    )
                bc_sb = work.tile([HD, 1024], FP32, tag="bc", bufs=1)
                nc.gpsimd.partition_broadcast(bc_sb[:], recip_row[:])
                nc.vector.tensor_tensor(
                    ct_tile[cpo : cpo + HD, q0 : q0 + 1024],
                    pc_ctx[0:HD, :],
                    bc_sb[:],
                    MULT,
                )
                result["lnrec"] = lnrec_cols

            def drive(gen, n):
                for _ in range(n):
                    if next(gen, "done") == "done":
                        return True
                return False

            units = [(h, qb) for h in range(HG) for qb in range(2)]
            results = [dict() for _ in units]
            gens = [b2_unit(h, qb, r) for (h, qb), r in zip(units, results)]
            # prologue: V projection interleaved with unit 0's B2 (V tiles
            # must stay ahead of the context matmuls consuming them)
            vg = v_gen()
            drive(vg, 8)
            for _ in range(8):
                drive(gens[0], 1)
                drive(vg, 1)
            drive(vg, 99)
            drive(gens[0], 9)

            for i, (h, qb) in enumerate(units):
                q0 = qb * 1024
                ti, po = h // 2, (h % 2) * HD
                qt_h = qt_t[ti][po : po + HD, :]
                kt_h = kt_t[ti][po : po + HD, :]
                # B1 qc=0 scores cover the fixup's recip/ln latency
                prev = b1_scores(qt_h, kt_h, q0, 0)
                drive(gens[i], 99)  # finish fixup of this unit
                lnrec_cols = results[i]["lnrec"]
                nxt_gen = gens[i + 1] if i + 1 < len(units) else None
                for qc in range(8):
                    nxt = b1_scores(qt_h, kt_h, q0, qc + 1) if qc < 7 else None
                    b1_expdma(h, q0, qc, prev, lnrec_cols[:, 2 * qc : 2 * qc + 1])
                    if nxt_gen is not None:
                        drive(nxt_gen, 2)  # interleave next unit's B2
                    prev = nxt
                # leave nxt_gen one step short of its fixup; the next loop
                # iteration emits its qc=0 scores first, then finishes it

            # ---- Stage C: output projection (partial) ----
            wot = [xw.tile([P, 512], FP32R, tag="wt", name="wot") for _ in range(8)]
            for pc in range(4):
                for oc in range(2):
                    nc.sync.dma_start(
                        wot[pc * 2 + oc][:],
                        wo[pc * P : (pc + 1) * P, oc * 512 : (oc + 1) * 512],
                    )
            for tk in range(16):
                for oc in range(2):
                    ps_o = psum_mm.tile([P, 512], FP32, tag="mm")
                    for pc in range(4):
                        nc.tensor.matmul(
                            ps_o[:],
                            ct_t[pc][:, tk * P : (tk + 1) * P],
                            wot[pc * 2 + oc][:],
                            start=(pc == 0),
                            stop=(pc == 3),
                        )
                    os_ = work.tile([P, 512], FP32, tag="attn", bufs=3, name="os_")
                    nc.scalar.copy(os_[:], ps_o[:])
                    nc.sync.dma_start(
                        out_p[tk * P : (tk + 1) * P, oc * 512 : (oc + 1) * 512], os_[:]
                    )

    nc.compile()
    return nc


_NC_CACHE = None


def _get_nc():
    global _NC_CACHE
    if _NC_CACHE is None:
        _NC_CACHE = build_nc()
    return _NC_CACHE


def _prep_in_maps(inputs):
    query = np.asarray(inputs["query"], np.float32)
    key = np.asarray(inputs["key"], np.float32)
    value = np.asarray(inputs["value"], np.float32)
    Wq = np.asarray(inputs["Wq"], np.float32)
    Wk = np.asarray(inputs["Wk"], np.float32)
    Wv = np.asarray(inputs["Wv"], np.float32)
    Wo = np.asarray(inputs["Wo"], np.float32)
    bq = np.asarray(inputs["bq"], np.float32)
    bk = np.asarray(inputs["bk"], np.float32)
    bv = np.asarray(inputs["bv"], np.float32)

    # per-batch and per-headgroup pieces are shared between cores
    xb = [
        {
            "xq": round_fp32r(query[b].T),
            "xk": round_fp32r(key[b].T),
            "xv": round_fp32r(value[b].T),
        }
        for b in range(B)
    ]
    ident = np.eye(P, dtype=np.float32)
    wg = []
    for g in range(2):
        sl = slice(g * PJ, (g + 1) * PJ)
        wg.append(
            {
                "wq": round_fp32r(np.ascontiguousarray(Wq[sl, :].T)),
                "wk": round_fp32r(np.ascontiguousarray(Wk[sl, :].T)),
                "wv": round_fp32r(np.ascontiguousarray(Wv[sl, :].T)),
                "wo": round_fp32r(np.ascontiguousarray(Wo[:, sl].T)),
                "bq": np.ascontiguousarray(bq[sl]).reshape(PJ, 1),
                "bk": np.ascontiguousarray(bk[sl]).reshape(PJ, 1),
                "bv": round_fp32r(bv[sl]).reshape(1, PJ),
                "ident": ident,
            }
        )
    return [{**xb[c // 2], **wg[c % 2]} for c in range(NCORES)]


def kernel(query, key, value, Wq, bq, Wk, bk, Wv, bv, Wo, bo, **_kw):
    bo = np.asarray(bo, np.float32)
    nc = _get_nc()
    in_maps = _prep_in_maps(
        dict(
            query=query, key=key, value=value, Wq=Wq, Wk=Wk, Wv=Wv, Wo=Wo,
            bq=bq, bk=bk, bv=bv,
        )
    )
    res = run_bass_kernel_spmd(nc, in_maps, core_ids=list(range(NCORES)))

    attn = np.empty((B, NH, S, S), np.float32)
    out = np.empty((B, S, D), np.float32)
    for b in range(B):
        r0, r1 = res.results[2 * b], res.results[2 * b + 1]
        attn[b, 0:HG] = r0["attn_o"]
        attn[b, HG:NH] = r1["attn_o"]
        out[b] = r0["out_p"] + r1["out_p"] + bo[None, :]
    return out, attn
